# revision 2
# baseline (speedup 1.0000x reference)
"""Causal self-attention (B=4, T=2048, C=1024, H=16) on 8 TRN2 NeuronCores.

Sharding: batch x head-halves. Core i handles batch b=i//2 and heads
[8*(i%2), 8*(i%2)+8). Each core computes QKV projection for its slice,
causal attention for its 8 heads, and a partial output projection
(512 of 1024 contraction features). The host sums the two partials per
batch and transposes back.

All matmuls run in float32r (TF32: fp32 storage, 10-bit-mantissa
products, fp32 accumulate) at 1 cycle/row -- ~4x faster than fp32 with
~1.5e-4 relative error. Inputs are pre-rounded to TF32 on the host so
the DMA'd bytes are already legal FP32R values.

Attention per head works on S^T tiles: S^T[k, q] so that the AV matmul
(lhsT = V [k, d+1], rhs = P^T [k, q]) needs no transposes, with a ones
column appended to V so row 64 of the PSUM accumulator collects the
softmax denominators. exp happens on the scalar engine PSUM->SBUF with
the 1/8 scale folded in; no max subtraction (scores are ~N(0,1), safe
in fp32). Causality: (k-tile, q-chunk) pairs above the diagonal are
skipped, diagonal tiles compute only the valid columns and get a
triangular mask multiply on the 128-wide diagonal block.
"""

import sys

if "/opt/trn_rl_repo" not in sys.path:
    sys.path.insert(0, "/opt/trn_rl_repo")

import numpy as np

import concourse.bass as bass
import concourse.mybir as mybir
import concourse.tile as tile
from concourse import bacc
from concourse.bass_utils import run_bass_kernel_spmd
from concourse.masks import make_upper_triangular

B, T, C, H = 4, 2048, 1024, 16
HD = C // H  # 64
NCORES = 8
HPC = H // 2  # heads per core = 8
F = HPC * HD  # 512 features per core
CH = 512  # t/q chunk width
NCH = T // CH  # 4
NKT = T // 128  # 16 k-tiles

f32 = mybir.dt.float32
f32r = mybir.dt.float32r


def build_nc():
    nc = bacc.Bacc("TRN2", target_bir_lowering=False, debug=False)
    xT = nc.dram_tensor("xT", [C, T], f32r, kind="ExternalInput").ap()
    wqk = nc.dram_tensor("wqk", [C, 2 * F], f32r, kind="ExternalInput").ap()
    wv = nc.dram_tensor("wv", [C, F], f32r, kind="ExternalInput").ap()
    wo = nc.dram_tensor("wo", [F, C], f32r, kind="ExternalInput").ap()
    yT = nc.dram_tensor("yT", [C, T], f32, kind="ExternalOutput").ap()

    with tile.TileContext(nc) as tc:
        with (
            tc.tile_pool(name="consts", bufs=1) as consts,
            tc.tile_pool(name="qkv", bufs=1) as qkv,
            tc.tile_pool(name="ps", bufs=4, space="PSUM") as ps,
            tc.tile_pool(name="pso", bufs=3, space="PSUM") as pso,
        ):
            tri = consts.tile([128, 128], f32)
            make_upper_triangular(nc, tri[:], val=1.0, diag=True)

            qt_sb = qkv.tile([128, 4, T], f32r)  # [pair, t], head 2p|2p+1 on parts 0-63|64-127
            kt_sb = qkv.tile([128, 4, T], f32r)
            v_sb = qkv.tile([128, NKT, HPC, HD + 1], f32r)  # V^T aug: ones at col HD
            ones = consts.tile([128, NKT * HPC], f32)
            nc.vector.memset(ones[:], 1.0)
            nc.vector.tensor_copy(
                out=v_sb[:, :, :, HD : HD + 1],
                in_=ones[:].rearrange("p (a b c) -> p a b c", a=NKT, b=HPC),
            )

            # ---- Phase 1: QKV projections ----
            with (
                tc.tile_pool(name="win", bufs=1) as win,
                tc.tile_pool(name="xin", bufs=2) as xin,
            ):
                wqk_sb = win.tile([128, 8, 2 * F], f32r)
                nc.sync.dma_start(
                    out=wqk_sb[:], in_=wqk.rearrange("(kt p) m -> p kt m", p=128)
                )
                wv_sb = win.tile([128, 8, F], f32r)
                nc.sync.dma_start(
                    out=wv_sb[:], in_=wv.rearrange("(kt p) m -> p kt m", p=128)
                )

                for j in range(NCH):
                    t0 = j * CH
                    xt = xin.tile([128, 8, CH], f32r, name=f"xt{j}", tag="xt")
                    nc.sync.dma_start(
                        out=xt[:],
                        in_=xT[:, t0 : t0 + CH].rearrange("(kt p) t -> p kt t", p=128),
                    )
                    # Q^T / K^T: out [m-tile 128, t-chunk 512], m 0-3 = Q pairs, 4-7 = K pairs
                    for m in range(8):
                        acc = ps.tile([128, CH], f32, name=f"qk_{j}_{m}", tag="mm")
                        for kt in range(8):
                            nc.tensor.matmul(
                                acc[:],
                                wqk_sb[:, kt, m * 128 : (m + 1) * 128],
                                xt[:, kt, :],
                                start=(kt == 0),
                                stop=(kt == 7),
                            )
                        dst = qt_sb if m < 4 else kt_sb
                        nc.vector.tensor_copy(
                            out=dst[:, m % 4, t0 : t0 + CH], in_=acc[:]
                        )
                    # V: out [t-subtile 128, 512 features], natural [t, f] layout
                    for s in range(CH // 128):
                        accv = ps.tile([128, F], f32, name=f"v_{j}_{s}", tag="mm")
                        for kt in range(8):
                            nc.tensor.matmul(
                                accv[:],
                                xt[:, kt, s * 128 : (s + 1) * 128],
                                wv_sb[:, kt, :],
                                start=(kt == 0),
                                stop=(kt == 7),
                            )
                        nc.vector.tensor_copy(
                            out=v_sb[:, 4 * j + s, :, 0:HD],
                            in_=accv[:].rearrange("p (h d) -> p h d", h=HPC),
                        )

            # ---- Phase 2: attention, Phase 3: output projection ----
            with (
                tc.tile_pool(name="wout", bufs=1) as wout,
                tc.tile_pool(name="obuf", bufs=1) as obuf,
                tc.tile_pool(name="pexp", bufs=4) as pexp,
                tc.tile_pool(name="bc", bufs=3) as bc,
                tc.tile_pool(name="tmpb", bufs=2) as tmpb,
                tc.tile_pool(name="ysb", bufs=3) as ysb,
                tc.tile_pool(name="drp", bufs=4, space="DRAM") as drp,
            ):
                wo_sb = wout.tile([128, 4, C], f32r)
                nc.sync.dma_start(
                    out=wo_sb[:], in_=wo.rearrange("(ft p) o -> p ft o", p=128)
                )
                o_sb = obuf.tile([128, 4, T], f32r)  # attn out^T [feature-tile, q]

                for j in range(NCH):
                    q0 = j * CH
                    nkt = 4 * j + 4
                    for hp in range(4):
                        o_ps = [
                            pso.tile(
                                [HD + 1, CH], f32, name=f"o_{j}_{hp}_{hf}", tag="o"
                            )
                            for hf in range(2)
                        ]
                        p_t = {}
                        for kt in range(nkt):
                            k0 = kt * 128
                            lo = max(k0 - q0, 0)
                            s_t = [
                                ps.tile(
                                    [128, CH], f32, name=f"s_{j}_{hp}_{kt}_{hf}",
                                    tag="mm",
                                )
                                for hf in range(2)
                            ]
                            p_t = [
                                pexp.tile(
                                    [128, CH], f32r, name=f"p_{j}_{hp}_{kt}_{hf}",
                                    tag="p",
                                )
                                for hf in range(2)
                            ]
                            # S^T = K^T.T @ Q^T for both heads of the pair;
                            # adjacent K=64 matmuls on row groups 0-63/64-127
                            # run concurrently in the PE array.
                            for hf in range(2):
                                pb = hf * 64
                                nc.tensor.matmul(
                                    s_t[hf][:, lo:CH],
                                    kt_sb[pb : pb + 64, hp, k0 : k0 + 128],
                                    qt_sb[pb : pb + 64, hp, q0 + lo : q0 + CH],
                                    start=True,
                                    stop=True,
                                )
                            for hf in range(2):
                                nc.scalar.activation(
                                    out=p_t[hf][:, lo:CH],
                                    in_=s_t[hf][:, lo:CH],
                                    func=mybir.ActivationFunctionType.Exp,
                                    scale=0.125,
                                )
                                if k0 >= q0:  # diagonal tile: triangular mask
                                    nc.vector.tensor_mul(
                                        out=p_t[hf][:, lo : lo + 128],
                                        in0=p_t[hf][:, lo : lo + 128],
                                        in1=tri[:],
                                    )
                            for hf in range(2):
                                h = 2 * hp + hf
                                nc.tensor.matmul(
                                    o_ps[hf][:, lo:CH],
                                    v_sb[:, kt, h, :],
                                    p_t[hf][:, lo:CH],
                                    start=(kt == 0),
                                    stop=(kt == nkt - 1),
                                )
                        # normalize: O^T[:, q] /= denom[q] (row HD of o_ps)
                        for hf in range(2):
                            recip = bc.tile(
                                [HD + 1, CH], f32, name=f"rc_{j}_{hp}_{hf}",
                                tag="recip",
                            )
                            nc.vector.reciprocal(
                                out=recip[HD : HD + 1, :], in_=o_ps[hf][HD : HD + 1, :]
                            )
                            bounce = drp.tile(
                                [CH], f32, name=f"bn_{j}_{hp}_{hf}", tag="bounce"
                            )
                            nc.sync.dma_start(
                                out=bounce[:], in_=recip[HD : HD + 1, :]
                            )
                            bcast = bc.tile(
                                [64, CH], f32, name=f"bb_{j}_{hp}_{hf}", tag="bcast"
                            )
                            nc.sync.dma_start(
                                out=bcast[:],
                                in_=bass.AP(
                                    tensor=bounce.tensor,
                                    offset=bounce.offset,
                                    ap=[[0, 64], [1, CH]],
                                ),
                            )
                            if hf == 0:
                                nc.vector.tensor_mul(
                                    out=o_sb[0:64, hp, q0 : q0 + CH],
                                    in0=o_ps[hf][0:HD, :],
                                    in1=bcast[:],
                                )
                            else:
                                # DVE can't shift partitions; write at base 0
                                # then DMA up to partitions 64-127.
                                tmp = tmpb.tile(
                                    [64, CH], f32r, name=f"tm_{j}_{hp}", tag="tm"
                                )
                                nc.vector.tensor_mul(
                                    out=tmp[:], in0=o_ps[hf][0:HD, :], in1=bcast[:]
                                )
                                nc.sync.dma_start(
                                    out=o_sb[64:128, hp, q0 : q0 + CH], in_=tmp[:]
                                )

                    # Phase 3 for this q-chunk
                    for ot in range(8):
                        acc = ps.tile([128, CH], f32, name=f"y_{j}_{ot}", tag="mm")
                        for ft in range(4):
                            nc.tensor.matmul(
                                acc[:],
                                wo_sb[:, ft, ot * 128 : (ot + 1) * 128],
                                o_sb[:, ft, q0 : q0 + CH],
                                start=(ft == 0),
                                stop=(ft == 3),
                            )
                        y = ysb.tile([128, CH], f32, name=f"ysb_{j}_{ot}", tag="y")
                        nc.vector.tensor_copy(out=y[:], in_=acc[:])
                        nc.sync.dma_start(
                            out=yT[ot * 128 : (ot + 1) * 128, q0 : q0 + CH], in_=y[:]
                        )

    nc.compile()
    return nc


def tf32_round(a: np.ndarray) -> np.ndarray:
    """Round fp32 to TF32 (10-bit mantissa), round-to-nearest-even."""
    a = np.ascontiguousarray(a, dtype=np.float32)
    u = a.view(np.uint32)
    r = (u + 0xFFF + ((u >> 13) & 1)) & np.uint32(0xFFFFE000)
    return r.astype(np.uint32).view(np.float32)


def shard_inputs(x, W_qkv, W_out):
    """Build the 8 per-core input maps."""
    xT = [tf32_round(np.ascontiguousarray(x[b].T)) for b in range(B)]
    maps = []
    for core in range(NCORES):
        b, hf = core // 2, core % 2
        wq = W_qkv[:, hf * F : (hf + 1) * F]
        wk = W_qkv[:, C + hf * F : C + (hf + 1) * F]
        wv = W_qkv[:, 2 * C + hf * F : 2 * C + (hf + 1) * F]
        maps.append(
            {
                "xT": xT[b],
                "wqk": tf32_round(np.concatenate([wq, wk], axis=1)),
                "wv": tf32_round(wv),
                "wo": tf32_round(W_out[hf * F : (hf + 1) * F, :]),
            }
        )
    return maps


_NC_CACHE = {}


def get_nc():
    if "nc" not in _NC_CACHE:
        _NC_CACHE["nc"] = build_nc()
    return _NC_CACHE["nc"]


def kernel(x, W_qkv, W_out, _run_kwargs=None):
    x = np.asarray(x, dtype=np.float32)
    W_qkv = np.asarray(W_qkv, dtype=np.float32)
    W_out = np.asarray(W_out, dtype=np.float32)
    nc = get_nc()
    maps = shard_inputs(x, W_qkv, W_out)
    res = run_bass_kernel_spmd(nc, maps, list(range(NCORES)), **(_run_kwargs or {}))
    out = np.empty((B, T, C), dtype=np.float32)
    for b in range(B):
        yT0 = res.results[2 * b]["yT"]
        yT1 = res.results[2 * b + 1]["yT"]
        out[b] = (yT0 + yT1).T
    if _run_kwargs is not None:
        _NC_CACHE["last_results"] = res
    return out


# revision 10
# speedup vs baseline: 1.2995x; 1.2995x over previous
"""Causal self-attention (B=4, T=2048, C=1024, H=16) on 8 TRN2 NeuronCores.

Sharding: batch x head-halves. Core i handles batch b=i//2 and heads
[8*(i%2), 8*(i%2)+8). Each core computes QKV projection for its slice,
causal attention for its 8 heads, and a partial output projection
(512 of 1024 contraction features). The host sums the two partials per
batch and transposes back.

All matmuls run in float32r (TF32: fp32 storage, 10-bit-mantissa
products, fp32 accumulate) at 1 cycle/row -- ~4x faster than fp32 with
~1.5e-4 relative error. Inputs are pre-rounded to TF32 on the host so
the DMA'd bytes are already legal FP32R values.

Attention per head works on S^T tiles: S^T[k, q] so that the AV matmul
(lhsT = V [k, d+1], rhs = P^T [k, q]) needs no transposes, with a ones
column appended to V so row 64 of the PSUM accumulator collects the
softmax denominators. exp happens on the scalar engine PSUM->SBUF with
the 1/8 scale folded in; no max subtraction (scores are ~N(0,1), safe
in fp32). Causality: (k-tile, q-chunk) pairs above the diagonal are
skipped, diagonal tiles compute only the valid columns and get a
triangular mask multiply on the 128-wide diagonal block.

Pipelining: phase1 (projections, per 512-t-chunk), phase2 (attention,
per 512-q-chunk) and phase3 (out-proj, one chunk behind) are emitted
interleaved -- legal because causal attention for q-chunk j only needs
K/V of chunks <= j. This keeps the PE stream dense (no HAM
re-throttle) and overlaps the scalar-engine exp work of phase2 with
projection matmuls. Softmax normalization happens out of the PE
critical path: the unnormalized O^T accumulator is copied to SBUF
immediately (freeing the PSUM bank), denominators of a whole chunk are
scattered via DRAM to [128,32] so the vector-engine reciprocal runs
parallel across lanes, and the per-head broadcasts come back as
partition-broadcast DMA reads.
"""

import sys

if "/opt/trn_rl_repo" not in sys.path:
    sys.path.insert(0, "/opt/trn_rl_repo")

import numpy as np

import concourse.bass as bass
import concourse.mybir as mybir
import concourse.tile as tile
from concourse import bacc
from concourse.bass_utils import run_bass_kernel_spmd
from concourse.masks import make_upper_triangular

B, T, C, H = 4, 2048, 1024, 16
HD = C // H  # 64
NCORES = 8
HPC = H // 2  # heads per core = 8
F = HPC * HD  # 512 features per core
CH = 512  # t/q chunk width
NCH = T // CH  # 4
NKT = T // 128  # 16 k-tiles

f32 = mybir.dt.float32
f32r = mybir.dt.float32r


def build_nc():
    nc = bacc.Bacc("TRN2", target_bir_lowering=False, debug=False)
    xT = nc.dram_tensor("xT", [C, T], f32r, kind="ExternalInput").ap()
    wqk = nc.dram_tensor("wqk", [C, 2 * F], f32r, kind="ExternalInput").ap()
    wv = nc.dram_tensor("wv", [C, F], f32r, kind="ExternalInput").ap()
    wo = nc.dram_tensor("wo", [F, C], f32r, kind="ExternalInput").ap()
    yT = nc.dram_tensor("yT", [C, T], f32, kind="ExternalOutput").ap()

    with tile.TileContext(nc) as tc:
        with (
            tc.tile_pool(name="consts", bufs=1) as consts,
            tc.tile_pool(name="kv", bufs=1) as kv,
            tc.tile_pool(name="qtp", bufs=2) as qtp,
            tc.tile_pool(name="win", bufs=1) as win,
            tc.tile_pool(name="xin", bufs=1) as xin,
            tc.tile_pool(name="wout", bufs=1) as wout,
            tc.tile_pool(name="obuf", bufs=2) as obuf,
            tc.tile_pool(name="oun", bufs=2) as oun,
            tc.tile_pool(name="pexp", bufs=3) as pexp,
            tc.tile_pool(name="bc", bufs=2) as bc,
            tc.tile_pool(name="tmpb", bufs=2) as tmpb,
            tc.tile_pool(name="ysb", bufs=2) as ysb,
            tc.tile_pool(name="drp", bufs=2, space="DRAM") as drp,
            tc.tile_pool(name="ps", bufs=4, space="PSUM") as ps,
            tc.tile_pool(name="pso", bufs=3, space="PSUM") as pso,
        ):
            tri = consts.tile([128, 128], f32)
            make_upper_triangular(nc, tri[:], val=1.0, diag=True)

            kt_sb = kv.tile([128, 4, T], f32r)  # K^T; head 2p|2p+1 on parts 0-63|64-127
            # V aug ([t, head, d] + ones col) in bf16: stationary operand of the
            # AV matmul only; its error passes linearly into O (~0.2% rel).
            v_sb = kv.tile([128, NKT, HPC, HD + 1], mybir.dt.bfloat16)
            ones = consts.tile([128, NKT * HPC], f32)
            nc.vector.memset(ones[:], 1.0)
            nc.vector.tensor_copy(
                out=v_sb[:, :, :, HD : HD + 1],
                in_=ones[:].rearrange("p (a b c) -> p a b c", a=NKT, b=HPC),
            )

            # weights: split DMAs per k-tile so the first matmuls start early
            xt_first = xin.tile([128, 8, CH], f32r, name="xt0", tag="xt")
            nc.sync.dma_start(
                out=xt_first[:], in_=xT[:, 0:CH].rearrange("(kt p) t -> p kt t", p=128)
            )
            wqk_sb = win.tile([128, 8, 2 * F], f32r)
            wv_sb = win.tile([128, 8, F], f32r)
            for kt in range(8):
                nc.sync.dma_start(
                    out=wqk_sb[:, kt, :], in_=wqk[kt * 128 : (kt + 1) * 128, :]
                )
            for kt in range(8):
                nc.sync.dma_start(
                    out=wv_sb[:, kt, :], in_=wv[kt * 128 : (kt + 1) * 128, :]
                )
            wo_sb = wout.tile([128, 4, C], f32r)
            nc.sync.dma_start(
                out=wo_sb[:], in_=wo.rearrange("(ft p) o -> p ft o", p=128)
            )

            qt_tiles = {}
            osb_tiles = {}

            def phase1(j):
                t0 = j * CH
                if j == 0:
                    xt = xt_first
                else:
                    xt = xin.tile([128, 8, CH], f32r, name=f"xt{j}", tag="xt")
                    nc.sync.dma_start(
                        out=xt[:],
                        in_=xT[:, t0 : t0 + CH].rearrange("(kt p) t -> p kt t", p=128),
                    )
                qt = qtp.tile([128, 4, CH], f32r, name=f"qt{j}", tag="qt")
                qt_tiles[j] = qt
                for m in range(8):
                    acc = ps.tile([128, CH], f32, name=f"qk_{j}_{m}", tag="mm")
                    for kt in range(8):
                        nc.tensor.matmul(
                            acc[:],
                            wqk_sb[:, kt, m * 128 : (m + 1) * 128],
                            xt[:, kt, :],
                            start=(kt == 0),
                            stop=(kt == 7),
                        )
                    if m < 4:
                        nc.vector.tensor_copy(out=qt[:, m, :], in_=acc[:])
                    else:
                        nc.vector.tensor_copy(
                            out=kt_sb[:, m % 4, t0 : t0 + CH], in_=acc[:]
                        )
                for s in range(CH // 128):
                    accv = ps.tile([128, F], f32, name=f"v_{j}_{s}", tag="mm")
                    for kt in range(8):
                        nc.tensor.matmul(
                            accv[:],
                            xt[:, kt, s * 128 : (s + 1) * 128],
                            wv_sb[:, kt, :],
                            start=(kt == 0),
                            stop=(kt == 7),
                        )
                    nc.vector.tensor_copy(
                        out=v_sb[:, 4 * j + s, :, 0:HD],
                        in_=accv[:].rearrange("p (h d) -> p h d", h=HPC),
                    )

            def phase2(j):
                q0 = j * CH
                nkt = 4 * j + 4
                o_sb = obuf.tile([128, 4, CH], f32r, name=f"osb{j}", tag="osb")
                osb_tiles[j] = o_sb
                for hp in range(4):
                    o_un = oun.tile(
                        [HD + 1, 2, CH], f32, name=f"oun_{j}_{hp}", tag="oun"
                    )
                    o_ps = [
                        pso.tile([HD + 1, CH], f32, name=f"o_{j}_{hp}_{hf}", tag="o")
                        for hf in range(2)
                    ]
                    for kt in range(nkt):
                        k0 = kt * 128
                        lo = max(k0 - q0, 0)
                        s_t = [
                            ps.tile(
                                [128, CH], f32, name=f"s_{j}_{hp}_{kt}_{hf}", tag="mm"
                            )
                            for hf in range(2)
                        ]
                        p_t = [
                            pexp.tile(
                                [128, CH], mybir.dt.bfloat16, name=f"p_{j}_{hp}_{kt}_{hf}", tag="p"
                            )
                            for hf in range(2)
                        ]
                        for hf in range(2):
                            pb = hf * 64
                            nc.tensor.matmul(
                                s_t[hf][:, lo:CH],
                                kt_sb[pb : pb + 64, hp, k0 : k0 + 128],
                                qt_tiles[j][pb : pb + 64, hp, lo:CH],
                                start=True,
                                stop=True,
                            )
                        for hf in range(2):
                            nc.scalar.activation(
                                out=p_t[hf][:, lo:CH],
                                in_=s_t[hf][:, lo:CH],
                                func=mybir.ActivationFunctionType.Exp,
                                scale=0.125,
                            )
                            if k0 >= q0:
                                nc.vector.tensor_mul(
                                    out=p_t[hf][:, lo : lo + 128],
                                    in0=p_t[hf][:, lo : lo + 128],
                                    in1=tri[:],
                                )
                        for hf in range(2):
                            h = 2 * hp + hf
                            nc.tensor.matmul(
                                o_ps[hf][:, lo:CH],
                                v_sb[:, kt, h, :],
                                p_t[hf][:, lo:CH],
                                start=(kt == 0),
                                stop=(kt == nkt - 1),
                            )
                    for hf in range(2):
                        nc.vector.tensor_copy(
                            out=o_un[:, hf, :], in_=o_ps[hf][:, :]
                        )
                    # normalize this head pair: scatter the 1024 denominators
                    # across 128 partitions via DRAM so reciprocal runs
                    # lane-parallel, then broadcast back per head.
                    dn = drp.tile([2 * CH], f32, name=f"dn{j}_{hp}", tag="dn")
                    nc.sync.dma_start(out=dn[:], in_=o_un[HD : HD + 1, :, :])
                    rc = drp.tile([2 * CH], f32, name=f"rc{j}_{hp}", tag="rc")
                    dsc = bc.tile([128, 8], f32, name=f"dsc{j}_{hp}", tag="dsc")
                    nc.sync.dma_start(
                        out=dsc[:], in_=dn[:].rearrange("(p c) -> p c", p=128)
                    )
                    nc.vector.reciprocal(out=dsc[:], in_=dsc[:])
                    nc.sync.dma_start(
                        out=rc[:].rearrange("(p c) -> p c", p=128), in_=dsc[:]
                    )
                    for hf in range(2):
                        bcast = bc.tile(
                            [64, CH], f32, name=f"bb_{j}_{hp}_{hf}", tag="bcast"
                        )
                        nc.sync.dma_start(
                            out=bcast[:],
                            in_=bass.AP(
                                tensor=rc.tensor,
                                offset=rc.offset + hf * CH,
                                ap=[[0, 64], [1, CH]],
                            ),
                        )
                        if hf == 0:
                            nc.vector.tensor_mul(
                                out=o_sb[0:64, hp, :],
                                in0=o_un[0:HD, 0, :],
                                in1=bcast[:],
                            )
                        else:
                            tmp = tmpb.tile(
                                [64, CH], f32r, name=f"tm_{j}_{hp}", tag="tm"
                            )
                            nc.vector.tensor_mul(
                                out=tmp[:], in0=o_un[0:HD, 1, :], in1=bcast[:]
                            )
                            nc.sync.dma_start(out=o_sb[64:128, hp, :], in_=tmp[:])

            def phase3(j):
                q0 = j * CH
                o_sb = osb_tiles[j]
                for ot in range(8):
                    acc = ps.tile([128, CH], f32, name=f"y_{j}_{ot}", tag="mm")
                    for ft in range(4):
                        nc.tensor.matmul(
                            acc[:],
                            wo_sb[:, ft, ot * 128 : (ot + 1) * 128],
                            o_sb[:, ft, :],
                            start=(ft == 0),
                            stop=(ft == 3),
                        )
                    y = ysb.tile([128, CH], f32, name=f"ysb_{j}_{ot}", tag="y")
                    nc.vector.tensor_copy(out=y[:], in_=acc[:])
                    nc.sync.dma_start(
                        out=yT[ot * 128 : (ot + 1) * 128, q0 : q0 + CH], in_=y[:]
                    )

            # interleaved emission; phase3 runs one chunk behind phase2 so the
            # normalization DMA pipeline hides under the next chunk's compute
            phase1(0)
            phase1(1)
            phase2(0)
            phase1(2)
            phase2(1)
            phase3(0)
            phase1(3)
            phase2(2)
            phase3(1)
            phase2(3)
            phase3(2)
            phase3(3)

    nc.compile()
    return nc


def tf32_round(a: np.ndarray) -> np.ndarray:
    """Round fp32 to TF32 (10-bit mantissa), round-to-nearest-even."""
    a = np.ascontiguousarray(a, dtype=np.float32)
    u = a.view(np.uint32)
    r = (u + 0xFFF + ((u >> 13) & 1)) & np.uint32(0xFFFFE000)
    return r.astype(np.uint32).view(np.float32)


def shard_inputs(x, W_qkv, W_out):
    """Build the 8 per-core input maps."""
    xT = [tf32_round(np.ascontiguousarray(x[b].T)) for b in range(B)]
    maps = []
    for core in range(NCORES):
        b, hf = core // 2, core % 2
        wq = W_qkv[:, hf * F : (hf + 1) * F]
        wk = W_qkv[:, C + hf * F : C + (hf + 1) * F]
        wv = W_qkv[:, 2 * C + hf * F : 2 * C + (hf + 1) * F]
        maps.append(
            {
                "xT": xT[b],
                "wqk": tf32_round(np.concatenate([wq, wk], axis=1)),
                "wv": tf32_round(wv),
                "wo": tf32_round(W_out[hf * F : (hf + 1) * F, :]),
            }
        )
    return maps


_NC_CACHE = {}


def get_nc():
    if "nc" not in _NC_CACHE:
        _NC_CACHE["nc"] = build_nc()
    return _NC_CACHE["nc"]


def kernel(x, W_qkv, W_out, _run_kwargs=None):
    x = np.asarray(x, dtype=np.float32)
    W_qkv = np.asarray(W_qkv, dtype=np.float32)
    W_out = np.asarray(W_out, dtype=np.float32)
    nc = get_nc()
    maps = shard_inputs(x, W_qkv, W_out)
    res = run_bass_kernel_spmd(nc, maps, list(range(NCORES)), **(_run_kwargs or {}))
    out = np.empty((B, T, C), dtype=np.float32)
    for b in range(B):
        yT0 = res.results[2 * b]["yT"]
        yT1 = res.results[2 * b + 1]["yT"]
        out[b] = (yT0 + yT1).T
    if _run_kwargs is not None:
        _NC_CACHE["last_results"] = res
    return out


# revision 12
# speedup vs baseline: 1.6324x; 1.2562x over previous
"""Causal self-attention (B=4, T=2048, C=1024, H=16) on 8 TRN2 NeuronCores.

Sharding: batch x head-halves. Core i handles batch b=i//2 and heads
[8*(i%2), 8*(i%2)+8). Each core computes QKV projection for its slice,
causal attention for its 8 heads, and a partial output projection
(512 of 1024 contraction features). The host sums the two partials per
batch and transposes back.

All matmuls run in float32r (TF32: fp32 storage, 10-bit-mantissa
products, fp32 accumulate) at 1 cycle/row -- ~4x faster than fp32 with
~1.5e-4 relative error. Inputs are pre-rounded to TF32 on the host so
the DMA'd bytes are already legal FP32R values.

Attention per head works on S^T tiles: S^T[k, q] so that the AV matmul
(lhsT = V [k, d+1], rhs = P^T [k, q]) needs no transposes, with a ones
column appended to V so row 64 of the PSUM accumulator collects the
softmax denominators. exp happens on the scalar engine PSUM->SBUF with
the 1/8 scale folded in; no max subtraction (scores are ~N(0,1), safe
in fp32). Causality: (k-tile, q-chunk) pairs above the diagonal are
skipped, diagonal tiles compute only the valid columns and get a
triangular mask multiply on the 128-wide diagonal block.

Pipelining: phase1 (projections, per 512-t-chunk), phase2 (attention,
per 512-q-chunk) and phase3 (out-proj, one chunk behind) are emitted
interleaved -- legal because causal attention for q-chunk j only needs
K/V of chunks <= j. This keeps the PE stream dense (no HAM
re-throttle) and overlaps the scalar-engine exp work of phase2 with
projection matmuls. Softmax normalization happens out of the PE
critical path: the unnormalized O^T accumulator is copied to SBUF
immediately (freeing the PSUM bank), denominators of a whole chunk are
scattered via DRAM to [128,32] so the vector-engine reciprocal runs
parallel across lanes, and the per-head broadcasts come back as
partition-broadcast DMA reads.
"""

import sys

if "/opt/trn_rl_repo" not in sys.path:
    sys.path.insert(0, "/opt/trn_rl_repo")

import numpy as np

import concourse.bass as bass
import concourse.mybir as mybir
import concourse.tile as tile
from concourse import bacc
from concourse.bass_utils import run_bass_kernel_spmd
from concourse.masks import make_upper_triangular

B, T, C, H = 4, 2048, 1024, 16
HD = C // H  # 64
NCORES = 8
HPC = H // 2  # heads per core = 8
F = HPC * HD  # 512 features per core
CH = 512  # t/q chunk width
NCH = T // CH  # 4
NKT = T // 128  # 16 k-tiles

f32 = mybir.dt.float32
f32r = mybir.dt.float32r


def build_nc():
    nc = bacc.Bacc("TRN2", target_bir_lowering=False, debug=False)
    xT = nc.dram_tensor("xT", [C, T], f32r, kind="ExternalInput").ap()
    wqk = nc.dram_tensor("wqk", [C, 2 * F], f32r, kind="ExternalInput").ap()
    wv = nc.dram_tensor("wv", [C, F], f32r, kind="ExternalInput").ap()
    wo = nc.dram_tensor("wo", [F, C], f32r, kind="ExternalInput").ap()
    yT = nc.dram_tensor("yT", [C, T], f32, kind="ExternalOutput").ap()

    with tile.TileContext(nc) as tc:
        with (
            tc.tile_pool(name="consts", bufs=1) as consts,
            tc.tile_pool(name="kv", bufs=1) as kv,
            tc.tile_pool(name="qtp", bufs=2) as qtp,
            tc.tile_pool(name="win", bufs=1) as win,
            tc.tile_pool(name="xin", bufs=1) as xin,
            tc.tile_pool(name="wout", bufs=1) as wout,
            tc.tile_pool(name="obuf", bufs=2) as obuf,
            tc.tile_pool(name="oun", bufs=2) as oun,
            tc.tile_pool(name="pexp", bufs=3) as pexp,
            tc.tile_pool(name="bc", bufs=2) as bc,
            tc.tile_pool(name="tmpb", bufs=2) as tmpb,
            tc.tile_pool(name="ysb", bufs=2) as ysb,
            tc.tile_pool(name="drp", bufs=2, space="DRAM") as drp,
            tc.tile_pool(name="ps", bufs=2, space="PSUM") as ps,
            tc.tile_pool(name="pss", bufs=2, space="PSUM") as pss,
            tc.tile_pool(name="pso", bufs=2, space="PSUM") as pso,
        ):
            tri = consts.tile([128, 128], f32)
            make_upper_triangular(nc, tri[:], val=1.0, diag=True)

            kt_sb = kv.tile([128, 4, T], f32r)  # K^T; head 2p|2p+1 on parts 0-63|64-127
            # V aug ([t, head, d] + ones col) in bf16: stationary operand of the
            # AV matmul only; its error passes linearly into O (~0.2% rel).
            v_sb = kv.tile([128, NKT, HPC, HD + 1], mybir.dt.bfloat16)
            ones = consts.tile([128, NKT * HPC], f32)
            nc.vector.memset(ones[:], 1.0)
            nc.vector.tensor_copy(
                out=v_sb[:, :, :, HD : HD + 1],
                in_=ones[:].rearrange("p (a b c) -> p a b c", a=NKT, b=HPC),
            )

            # weights: split DMAs per k-tile so the first matmuls start early
            xt_first = xin.tile([128, 8, CH], f32r, name="xt0", tag="xt")
            nc.sync.dma_start(
                out=xt_first[:], in_=xT[:, 0:CH].rearrange("(kt p) t -> p kt t", p=128)
            )
            wqk_sb = win.tile([128, 8, 2 * F], f32r)
            wv_sb = win.tile([128, 8, F], f32r)
            for kt in range(8):
                nc.sync.dma_start(
                    out=wqk_sb[:, kt, :], in_=wqk[kt * 128 : (kt + 1) * 128, :]
                )
            for kt in range(8):
                nc.sync.dma_start(
                    out=wv_sb[:, kt, :], in_=wv[kt * 128 : (kt + 1) * 128, :]
                )
            wo_sb = wout.tile([128, 4, C], f32r)
            nc.sync.dma_start(
                out=wo_sb[:], in_=wo.rearrange("(ft p) o -> p ft o", p=128)
            )

            qt_tiles = {}
            osb_tiles = {}

            def phase1(j):
                t0 = j * CH
                if j == 0:
                    xt = xt_first
                else:
                    xt = xin.tile([128, 8, CH], f32r, name=f"xt{j}", tag="xt")
                    nc.sync.dma_start(
                        out=xt[:],
                        in_=xT[:, t0 : t0 + CH].rearrange("(kt p) t -> p kt t", p=128),
                    )
                qt = qtp.tile([128, 4, CH], f32r, name=f"qt{j}", tag="qt")
                qt_tiles[j] = qt
                for m in range(8):
                    acc = ps.tile([128, CH], f32, name=f"qk_{j}_{m}", tag="mm")
                    for kt in range(8):
                        nc.tensor.matmul(
                            acc[:],
                            wqk_sb[:, kt, m * 128 : (m + 1) * 128],
                            xt[:, kt, :],
                            start=(kt == 0),
                            stop=(kt == 7),
                        )
                    if m < 4:
                        nc.vector.tensor_copy(out=qt[:, m, :], in_=acc[:])
                    else:
                        nc.vector.tensor_copy(
                            out=kt_sb[:, m % 4, t0 : t0 + CH], in_=acc[:]
                        )
                for s in range(CH // 128):
                    accv = ps.tile([128, F], f32, name=f"v_{j}_{s}", tag="mm")
                    for kt in range(8):
                        nc.tensor.matmul(
                            accv[:],
                            xt[:, kt, s * 128 : (s + 1) * 128],
                            wv_sb[:, kt, :],
                            start=(kt == 0),
                            stop=(kt == 7),
                        )
                    nc.vector.tensor_copy(
                        out=v_sb[:, 4 * j + s, :, 0:HD],
                        in_=accv[:].rearrange("p (h d) -> p h d", h=HPC),
                    )

            def phase2(j):
                q0 = j * CH
                nkt = 4 * j + 4
                o_sb = obuf.tile([128, 4, CH], f32r, name=f"osb{j}", tag="osb")
                osb_tiles[j] = o_sb
                for hp in range(4):
                    o_un = oun.tile(
                        [HD + 1, 2, CH], f32, name=f"oun_{j}_{hp}", tag="oun"
                    )
                    o_ps = [
                        pso.tile([HD + 1, CH], f32, name=f"o_{j}_{hp}_{hf}", tag="o")
                        for hf in range(2)
                    ]
                    def av(kt, p_t, lo):
                        for hf in range(2):
                            h = 2 * hp + hf
                            nc.tensor.matmul(
                                o_ps[hf][:, lo:CH],
                                v_sb[:, kt, h, :],
                                p_t[:, hf, lo:CH],
                                start=(kt == 0),
                                stop=(kt == nkt - 1),
                            )

                    prev = None
                    for kt in range(nkt):
                        k0 = kt * 128
                        lo = max(k0 - q0, 0)
                        # both heads' scores in one 2-bank PSUM tile so a
                        # single wide exp covers the pair; the two K=64
                        # matmuls sit on disjoint PE row groups (0-63 /
                        # 64-127) and are emitted back-to-back so they run
                        # concurrently in the array.
                        s_t = pss.tile(
                            [128, 2, CH], f32, name=f"s_{j}_{hp}_{kt}", tag="s"
                        )
                        p_t = pexp.tile(
                            [128, 2, CH],
                            mybir.dt.bfloat16,
                            name=f"p_{j}_{hp}_{kt}",
                            tag="p",
                        )
                        for hf in range(2):
                            pb = hf * 64
                            nc.tensor.matmul(
                                s_t[:, hf, lo:CH],
                                kt_sb[pb : pb + 64, hp, k0 : k0 + 128],
                                qt_tiles[j][pb : pb + 64, hp, lo:CH],
                                start=True,
                                stop=True,
                            )
                        if lo == 0:
                            nc.scalar.activation(
                                out=p_t[:, :, :],
                                in_=s_t[:, :, :],
                                func=mybir.ActivationFunctionType.Exp,
                                scale=0.125,
                            )
                        else:
                            for hf in range(2):
                                nc.scalar.activation(
                                    out=p_t[:, hf, lo:CH],
                                    in_=s_t[:, hf, lo:CH],
                                    func=mybir.ActivationFunctionType.Exp,
                                    scale=0.125,
                                )
                        if k0 >= q0:
                            for hf in range(2):
                                nc.vector.tensor_mul(
                                    out=p_t[:, hf, lo : lo + 128],
                                    in0=p_t[:, hf, lo : lo + 128],
                                    in1=tri[:],
                                )
                        # AV runs one k-tile behind so exp(kt) overlaps it
                        if prev is not None:
                            av(*prev)
                        prev = (kt, p_t, lo)
                    av(*prev)
                    for hf in range(2):
                        nc.vector.tensor_copy(
                            out=o_un[:, hf, :], in_=o_ps[hf][:, :]
                        )
                    # normalize this head pair: scatter the 1024 denominators
                    # across 128 partitions via DRAM so reciprocal runs
                    # lane-parallel, then broadcast back per head.
                    dn = drp.tile([2 * CH], f32, name=f"dn{j}_{hp}", tag="dn")
                    nc.sync.dma_start(out=dn[:], in_=o_un[HD : HD + 1, :, :])
                    rc = drp.tile([2 * CH], f32, name=f"rc{j}_{hp}", tag="rc")
                    dsc = bc.tile([128, 8], f32, name=f"dsc{j}_{hp}", tag="dsc")
                    nc.sync.dma_start(
                        out=dsc[:], in_=dn[:].rearrange("(p c) -> p c", p=128)
                    )
                    nc.vector.reciprocal(out=dsc[:], in_=dsc[:])
                    nc.sync.dma_start(
                        out=rc[:].rearrange("(p c) -> p c", p=128), in_=dsc[:]
                    )
                    for hf in range(2):
                        bcast = bc.tile(
                            [64, CH], f32, name=f"bb_{j}_{hp}_{hf}", tag="bcast"
                        )
                        nc.sync.dma_start(
                            out=bcast[:],
                            in_=bass.AP(
                                tensor=rc.tensor,
                                offset=rc.offset + hf * CH,
                                ap=[[0, 64], [1, CH]],
                            ),
                        )
                        if hf == 0:
                            nc.vector.tensor_mul(
                                out=o_sb[0:64, hp, :],
                                in0=o_un[0:HD, 0, :],
                                in1=bcast[:],
                            )
                        else:
                            tmp = tmpb.tile(
                                [64, CH], f32r, name=f"tm_{j}_{hp}", tag="tm"
                            )
                            nc.vector.tensor_mul(
                                out=tmp[:], in0=o_un[0:HD, 1, :], in1=bcast[:]
                            )
                            nc.sync.dma_start(out=o_sb[64:128, hp, :], in_=tmp[:])

            def phase3(j):
                q0 = j * CH
                o_sb = osb_tiles[j]
                for ot in range(8):
                    acc = ps.tile([128, CH], f32, name=f"y_{j}_{ot}", tag="mm")
                    for ft in range(4):
                        nc.tensor.matmul(
                            acc[:],
                            wo_sb[:, ft, ot * 128 : (ot + 1) * 128],
                            o_sb[:, ft, :],
                            start=(ft == 0),
                            stop=(ft == 3),
                        )
                    y = ysb.tile([128, CH], f32, name=f"ysb_{j}_{ot}", tag="y")
                    nc.vector.tensor_copy(out=y[:], in_=acc[:])
                    nc.sync.dma_start(
                        out=yT[ot * 128 : (ot + 1) * 128, q0 : q0 + CH], in_=y[:]
                    )

            # interleaved emission; phase3 runs one chunk behind phase2 so the
            # normalization DMA pipeline hides under the next chunk's compute
            phase1(0)
            phase1(1)
            phase2(0)
            phase1(2)
            phase2(1)
            phase3(0)
            phase1(3)
            phase2(2)
            phase3(1)
            phase2(3)
            phase3(2)
            phase3(3)

    nc.compile()
    return nc


def tf32_round(a: np.ndarray) -> np.ndarray:
    """Round fp32 to TF32 (10-bit mantissa), round-to-nearest-even."""
    a = np.ascontiguousarray(a, dtype=np.float32)
    u = a.view(np.uint32)
    r = (u + 0xFFF + ((u >> 13) & 1)) & np.uint32(0xFFFFE000)
    return r.astype(np.uint32).view(np.float32)


def shard_inputs(x, W_qkv, W_out):
    """Build the 8 per-core input maps."""
    xT = [tf32_round(np.ascontiguousarray(x[b].T)) for b in range(B)]
    maps = []
    for core in range(NCORES):
        b, hf = core // 2, core % 2
        wq = W_qkv[:, hf * F : (hf + 1) * F]
        wk = W_qkv[:, C + hf * F : C + (hf + 1) * F]
        wv = W_qkv[:, 2 * C + hf * F : 2 * C + (hf + 1) * F]
        maps.append(
            {
                "xT": xT[b],
                "wqk": tf32_round(np.concatenate([wq, wk], axis=1)),
                "wv": tf32_round(wv),
                "wo": tf32_round(W_out[hf * F : (hf + 1) * F, :]),
            }
        )
    return maps


_NC_CACHE = {}


def get_nc():
    if "nc" not in _NC_CACHE:
        _NC_CACHE["nc"] = build_nc()
    return _NC_CACHE["nc"]


def kernel(x, W_qkv, W_out, _run_kwargs=None):
    x = np.asarray(x, dtype=np.float32)
    W_qkv = np.asarray(W_qkv, dtype=np.float32)
    W_out = np.asarray(W_out, dtype=np.float32)
    nc = get_nc()
    maps = shard_inputs(x, W_qkv, W_out)
    res = run_bass_kernel_spmd(nc, maps, list(range(NCORES)), **(_run_kwargs or {}))
    out = np.empty((B, T, C), dtype=np.float32)
    for b in range(B):
        yT0 = res.results[2 * b]["yT"]
        yT1 = res.results[2 * b + 1]["yT"]
        out[b] = (yT0 + yT1).T
    if _run_kwargs is not None:
        _NC_CACHE["last_results"] = res
    return out


# revision 14
# speedup vs baseline: 1.6332x; 1.0005x over previous
"""Causal self-attention (B=4, T=2048, C=1024, H=16) on 8 TRN2 NeuronCores.

Sharding: batch x head-halves. Core i handles batch b=i//2 and heads
[8*(i%2), 8*(i%2)+8). Each core computes QKV projection for its slice,
causal attention for its 8 heads, and a partial output projection
(512 of 1024 contraction features). The host sums the two partials per
batch and transposes back.

All matmuls run in float32r (TF32: fp32 storage, 10-bit-mantissa
products, fp32 accumulate) at 1 cycle/row -- ~4x faster than fp32 with
~1.5e-4 relative error. Inputs are pre-rounded to TF32 on the host so
the DMA'd bytes are already legal FP32R values.

Attention per head works on S^T tiles: S^T[k, q] so that the AV matmul
(lhsT = V [k, d+1], rhs = P^T [k, q]) needs no transposes, with a ones
column appended to V so row 64 of the PSUM accumulator collects the
softmax denominators. exp happens on the scalar engine PSUM->SBUF with
the 1/8 scale folded in; no max subtraction (scores are ~N(0,1), safe
in fp32). Causality: (k-tile, q-chunk) pairs above the diagonal are
skipped, diagonal tiles compute only the valid columns and get a
triangular mask multiply on the 128-wide diagonal block.

Pipelining: phase1 (projections, per 512-t-chunk), phase2 (attention,
per 512-q-chunk) and phase3 (out-proj, one chunk behind) are emitted
interleaved -- legal because causal attention for q-chunk j only needs
K/V of chunks <= j. This keeps the PE stream dense (no HAM
re-throttle) and overlaps the scalar-engine exp work of phase2 with
projection matmuls. Softmax normalization happens out of the PE
critical path: the unnormalized O^T accumulator is copied to SBUF
immediately (freeing the PSUM bank), denominators of a whole chunk are
scattered via DRAM to [128,32] so the vector-engine reciprocal runs
parallel across lanes, and the per-head broadcasts come back as
partition-broadcast DMA reads.
"""

import sys

if "/opt/trn_rl_repo" not in sys.path:
    sys.path.insert(0, "/opt/trn_rl_repo")

import numpy as np

import concourse.bass as bass
import concourse.mybir as mybir
import concourse.tile as tile
from concourse import bacc
from concourse.bass_utils import run_bass_kernel_spmd
from concourse.masks import make_upper_triangular

B, T, C, H = 4, 2048, 1024, 16
HD = C // H  # 64
NCORES = 8
HPC = H // 2  # heads per core = 8
F = HPC * HD  # 512 features per core
CH = 512  # t/q chunk width
NCH = T // CH  # 4
NKT = T // 128  # 16 k-tiles

f32 = mybir.dt.float32
f32r = mybir.dt.float32r


def build_nc():
    nc = bacc.Bacc("TRN2", target_bir_lowering=False, debug=False)
    xT = nc.dram_tensor("xT", [C, T], f32r, kind="ExternalInput").ap()
    wqk = nc.dram_tensor("wqk", [C, 2 * F], f32r, kind="ExternalInput").ap()
    wv = nc.dram_tensor("wv", [C, F], f32r, kind="ExternalInput").ap()
    wo = nc.dram_tensor("wo", [F, C], f32r, kind="ExternalInput").ap()
    yT = nc.dram_tensor("yT", [C, T], f32, kind="ExternalOutput").ap()

    with tile.TileContext(nc) as tc:
        with (
            tc.tile_pool(name="consts", bufs=1) as consts,
            tc.tile_pool(name="kv", bufs=1) as kv,
            tc.tile_pool(name="qtp", bufs=2) as qtp,
            tc.tile_pool(name="win", bufs=1) as win,
            tc.tile_pool(name="xin", bufs=1) as xin,
            tc.tile_pool(name="wout", bufs=1) as wout,
            tc.tile_pool(name="obuf", bufs=2) as obuf,
            tc.tile_pool(name="oun", bufs=2) as oun,
            tc.tile_pool(name="pexp", bufs=3) as pexp,
            tc.tile_pool(name="bc", bufs=2) as bc,
            tc.tile_pool(name="tmpb", bufs=2) as tmpb,
            tc.tile_pool(name="ysb", bufs=2) as ysb,
            tc.tile_pool(name="drp", bufs=2, space="DRAM") as drp,
            tc.tile_pool(name="ps", bufs=2, space="PSUM") as ps,
            tc.tile_pool(name="pss", bufs=2, space="PSUM") as pss,
            tc.tile_pool(name="pso", bufs=2, space="PSUM") as pso,
        ):
            tri = consts.tile([128, 128], f32)
            make_upper_triangular(nc, tri[:], val=1.0, diag=True)

            kt_sb = kv.tile([128, 4, T], f32r)  # K^T; head 2p|2p+1 on parts 0-63|64-127
            # V aug ([t, head, d] + ones col) in bf16: stationary operand of the
            # AV matmul only; its error passes linearly into O (~0.2% rel).
            v_sb = kv.tile([128, NKT, HPC, HD + 1], mybir.dt.bfloat16)
            ones = consts.tile([128, NKT * HPC], f32)
            nc.vector.memset(ones[:], 1.0)
            nc.vector.tensor_copy(
                out=v_sb[:, :, :, HD : HD + 1],
                in_=ones[:].rearrange("p (a b c) -> p a b c", a=NKT, b=HPC),
            )

            # weights: split DMAs per k-tile so the first matmuls start early
            xt_first = xin.tile([128, 8, CH], f32r, name="xt0", tag="xt")
            for kt in range(8):
                nc.sync.dma_start(
                    out=xt_first[:, kt, :],
                    in_=xT[kt * 128 : (kt + 1) * 128, 0:CH],
                )
            wqk_sb = win.tile([128, 8, 2 * F], f32r)
            wv_sb = win.tile([128, 8, F], f32r)
            for kt in range(8):
                nc.sync.dma_start(
                    out=wqk_sb[:, kt, :], in_=wqk[kt * 128 : (kt + 1) * 128, :]
                )
            for kt in range(8):
                nc.sync.dma_start(
                    out=wv_sb[:, kt, :], in_=wv[kt * 128 : (kt + 1) * 128, :]
                )
            wo_sb = wout.tile([128, 4, C], f32r)
            nc.sync.dma_start(
                out=wo_sb[:], in_=wo.rearrange("(ft p) o -> p ft o", p=128)
            )

            qt_tiles = {}
            osb_tiles = {}

            def phase1(j):
                t0 = j * CH
                if j == 0:
                    xt = xt_first
                else:
                    xt = xin.tile([128, 8, CH], f32r, name=f"xt{j}", tag="xt")
                    nc.sync.dma_start(
                        out=xt[:],
                        in_=xT[:, t0 : t0 + CH].rearrange("(kt p) t -> p kt t", p=128),
                    )
                qt = qtp.tile([128, 4, CH], f32r, name=f"qt{j}", tag="qt")
                qt_tiles[j] = qt
                for m in range(8):
                    acc = ps.tile([128, CH], f32, name=f"qk_{j}_{m}", tag="mm")
                    for kt in range(8):
                        nc.tensor.matmul(
                            acc[:],
                            wqk_sb[:, kt, m * 128 : (m + 1) * 128],
                            xt[:, kt, :],
                            start=(kt == 0),
                            stop=(kt == 7),
                        )
                    if m < 4:
                        nc.vector.tensor_copy(out=qt[:, m, :], in_=acc[:])
                    else:
                        nc.vector.tensor_copy(
                            out=kt_sb[:, m % 4, t0 : t0 + CH], in_=acc[:]
                        )
                for s in range(CH // 128):
                    accv = ps.tile([128, F], f32, name=f"v_{j}_{s}", tag="mm")
                    for kt in range(8):
                        nc.tensor.matmul(
                            accv[:],
                            xt[:, kt, s * 128 : (s + 1) * 128],
                            wv_sb[:, kt, :],
                            start=(kt == 0),
                            stop=(kt == 7),
                        )
                    nc.vector.tensor_copy(
                        out=v_sb[:, 4 * j + s, :, 0:HD],
                        in_=accv[:].rearrange("p (h d) -> p h d", h=HPC),
                    )

            def phase2(j):
                q0 = j * CH
                nkt = 4 * j + 4
                o_sb = obuf.tile([128, 4, CH], f32r, name=f"osb{j}", tag="osb")
                osb_tiles[j] = o_sb
                for hp in range(4):
                    o_un = oun.tile(
                        [HD + 1, 2, CH], f32, name=f"oun_{j}_{hp}", tag="oun"
                    )
                    o_ps = [
                        pso.tile([HD + 1, CH], f32, name=f"o_{j}_{hp}_{hf}", tag="o")
                        for hf in range(2)
                    ]
                    def av(kt, p_t, lo):
                        for hf in range(2):
                            h = 2 * hp + hf
                            nc.tensor.matmul(
                                o_ps[hf][:, lo:CH],
                                v_sb[:, kt, h, :],
                                p_t[:, hf, lo:CH],
                                start=(kt == 0),
                                stop=(kt == nkt - 1),
                            )

                    prev = None
                    for kt in range(nkt):
                        k0 = kt * 128
                        lo = max(k0 - q0, 0)
                        # both heads' scores in one 2-bank PSUM tile so a
                        # single wide exp covers the pair; the two K=64
                        # matmuls sit on disjoint PE row groups (0-63 /
                        # 64-127) and are emitted back-to-back so they run
                        # concurrently in the array.
                        s_t = pss.tile(
                            [128, 2, CH], f32, name=f"s_{j}_{hp}_{kt}", tag="s"
                        )
                        p_t = pexp.tile(
                            [128, 2, CH],
                            mybir.dt.bfloat16,
                            name=f"p_{j}_{hp}_{kt}",
                            tag="p",
                        )
                        for hf in range(2):
                            pb = hf * 64
                            nc.tensor.matmul(
                                s_t[:, hf, lo:CH],
                                kt_sb[pb : pb + 64, hp, k0 : k0 + 128],
                                qt_tiles[j][pb : pb + 64, hp, lo:CH],
                                start=True,
                                stop=True,
                            )
                        nc.scalar.activation(
                            out=p_t[:, :, lo:CH],
                            in_=s_t[:, :, lo:CH],
                            func=mybir.ActivationFunctionType.Exp,
                            scale=0.125,
                        )
                        if k0 >= q0:
                            for hf in range(2):
                                nc.vector.tensor_mul(
                                    out=p_t[:, hf, lo : lo + 128],
                                    in0=p_t[:, hf, lo : lo + 128],
                                    in1=tri[:],
                                )
                        # AV runs one k-tile behind so exp(kt) overlaps it
                        if prev is not None:
                            av(*prev)
                        prev = (kt, p_t, lo)
                    av(*prev)
                    for hf in range(2):
                        nc.vector.tensor_copy(
                            out=o_un[:, hf, :], in_=o_ps[hf][:, :]
                        )
                    # normalize this head pair: scatter the 1024 denominators
                    # across 128 partitions via DRAM so reciprocal runs
                    # lane-parallel, then broadcast back per head.
                    dn = drp.tile([2 * CH], f32, name=f"dn{j}_{hp}", tag="dn")
                    nc.sync.dma_start(out=dn[:], in_=o_un[HD : HD + 1, :, :])
                    rc = drp.tile([2 * CH], f32, name=f"rc{j}_{hp}", tag="rc")
                    dsc = bc.tile([128, 8], f32, name=f"dsc{j}_{hp}", tag="dsc")
                    nc.sync.dma_start(
                        out=dsc[:], in_=dn[:].rearrange("(p c) -> p c", p=128)
                    )
                    nc.vector.reciprocal(out=dsc[:], in_=dsc[:])
                    nc.sync.dma_start(
                        out=rc[:].rearrange("(p c) -> p c", p=128), in_=dsc[:]
                    )
                    for hf in range(2):
                        bcast = bc.tile(
                            [64, CH], f32, name=f"bb_{j}_{hp}_{hf}", tag="bcast"
                        )
                        nc.sync.dma_start(
                            out=bcast[:],
                            in_=bass.AP(
                                tensor=rc.tensor,
                                offset=rc.offset + hf * CH,
                                ap=[[0, 64], [1, CH]],
                            ),
                        )
                        if hf == 0:
                            nc.vector.tensor_mul(
                                out=o_sb[0:64, hp, :],
                                in0=o_un[0:HD, 0, :],
                                in1=bcast[:],
                            )
                        else:
                            tmp = tmpb.tile(
                                [64, CH], f32r, name=f"tm_{j}_{hp}", tag="tm"
                            )
                            nc.vector.tensor_mul(
                                out=tmp[:], in0=o_un[0:HD, 1, :], in1=bcast[:]
                            )
                            nc.sync.dma_start(out=o_sb[64:128, hp, :], in_=tmp[:])

            def phase3(j):
                q0 = j * CH
                o_sb = osb_tiles[j]
                for ot in range(8):
                    acc = ps.tile([128, CH], f32, name=f"y_{j}_{ot}", tag="mm")
                    for ft in range(4):
                        nc.tensor.matmul(
                            acc[:],
                            wo_sb[:, ft, ot * 128 : (ot + 1) * 128],
                            o_sb[:, ft, :],
                            start=(ft == 0),
                            stop=(ft == 3),
                        )
                    y = ysb.tile([128, CH], f32, name=f"ysb_{j}_{ot}", tag="y")
                    nc.vector.tensor_copy(out=y[:], in_=acc[:])
                    nc.sync.dma_start(
                        out=yT[ot * 128 : (ot + 1) * 128, q0 : q0 + CH], in_=y[:]
                    )

            # interleaved emission; phase3 runs one chunk behind phase2 so the
            # normalization DMA pipeline hides under the next chunk's compute
            phase1(0)
            phase1(1)
            phase2(0)
            phase1(2)
            phase2(1)
            phase3(0)
            phase1(3)
            phase2(2)
            phase3(1)
            phase2(3)
            phase3(2)
            phase3(3)

    nc.compile()
    return nc


def tf32_round(a: np.ndarray) -> np.ndarray:
    """Round fp32 to TF32 (10-bit mantissa), round-to-nearest-even."""
    a = np.ascontiguousarray(a, dtype=np.float32)
    u = a.view(np.uint32)
    r = (u + 0xFFF + ((u >> 13) & 1)) & np.uint32(0xFFFFE000)
    return r.astype(np.uint32).view(np.float32)


def shard_inputs(x, W_qkv, W_out):
    """Build the 8 per-core input maps."""
    xT = [tf32_round(np.ascontiguousarray(x[b].T)) for b in range(B)]
    maps = []
    for core in range(NCORES):
        b, hf = core // 2, core % 2
        wq = W_qkv[:, hf * F : (hf + 1) * F]
        wk = W_qkv[:, C + hf * F : C + (hf + 1) * F]
        wv = W_qkv[:, 2 * C + hf * F : 2 * C + (hf + 1) * F]
        maps.append(
            {
                "xT": xT[b],
                "wqk": tf32_round(np.concatenate([wq, wk], axis=1)),
                "wv": tf32_round(wv),
                "wo": tf32_round(W_out[hf * F : (hf + 1) * F, :]),
            }
        )
    return maps


_NC_CACHE = {}


def get_nc():
    if "nc" not in _NC_CACHE:
        _NC_CACHE["nc"] = build_nc()
    return _NC_CACHE["nc"]


def kernel(x, W_qkv, W_out, _run_kwargs=None):
    x = np.asarray(x, dtype=np.float32)
    W_qkv = np.asarray(W_qkv, dtype=np.float32)
    W_out = np.asarray(W_out, dtype=np.float32)
    nc = get_nc()
    maps = shard_inputs(x, W_qkv, W_out)
    res = run_bass_kernel_spmd(nc, maps, list(range(NCORES)), **(_run_kwargs or {}))
    out = np.empty((B, T, C), dtype=np.float32)
    for b in range(B):
        yT0 = res.results[2 * b]["yT"]
        yT1 = res.results[2 * b + 1]["yT"]
        out[b] = (yT0 + yT1).T
    if _run_kwargs is not None:
        _NC_CACHE["last_results"] = res
    return out


# revision 16
# speedup vs baseline: 1.6534x; 1.0124x over previous
"""Causal self-attention (B=4, T=2048, C=1024, H=16) on 8 TRN2 NeuronCores.

Sharding: batch x head-halves. Core i handles batch b=i//2 and heads
[8*(i%2), 8*(i%2)+8). Each core computes QKV projection for its slice,
causal attention for its 8 heads, and a partial output projection
(512 of 1024 contraction features). The host sums the two partials per
batch and transposes back.

All matmuls run in float32r (TF32: fp32 storage, 10-bit-mantissa
products, fp32 accumulate) at 1 cycle/row -- ~4x faster than fp32 with
~1.5e-4 relative error. Inputs are pre-rounded to TF32 on the host so
the DMA'd bytes are already legal FP32R values.

Attention per head works on S^T tiles: S^T[k, q] so that the AV matmul
(lhsT = V [k, d+1], rhs = P^T [k, q]) needs no transposes, with a ones
column appended to V so row 64 of the PSUM accumulator collects the
softmax denominators. exp happens on the scalar engine PSUM->SBUF with
the 1/8 scale folded in; no max subtraction (scores are ~N(0,1), safe
in fp32). Causality: (k-tile, q-chunk) pairs above the diagonal are
skipped, diagonal tiles compute only the valid columns and get a
triangular mask multiply on the 128-wide diagonal block.

Pipelining: phase1 (projections, per 512-t-chunk), phase2 (attention,
per 512-q-chunk) and phase3 (out-proj, one chunk behind) are emitted
interleaved -- legal because causal attention for q-chunk j only needs
K/V of chunks <= j. This keeps the PE stream dense (no HAM
re-throttle) and overlaps the scalar-engine exp work of phase2 with
projection matmuls. Softmax normalization happens out of the PE
critical path: the unnormalized O^T accumulator is copied to SBUF
immediately (freeing the PSUM bank), denominators of a whole chunk are
scattered via DRAM to [128,32] so the vector-engine reciprocal runs
parallel across lanes, and the per-head broadcasts come back as
partition-broadcast DMA reads.
"""

import sys

if "/opt/trn_rl_repo" not in sys.path:
    sys.path.insert(0, "/opt/trn_rl_repo")

import numpy as np

import concourse.bass as bass
import concourse.mybir as mybir
import concourse.tile as tile
from concourse import bacc
from concourse.bass_utils import run_bass_kernel_spmd
from concourse.masks import make_upper_triangular

B, T, C, H = 4, 2048, 1024, 16
HD = C // H  # 64
NCORES = 8
HPC = H // 2  # heads per core = 8
F = HPC * HD  # 512 features per core
CH = 512  # t/q chunk width
NCH = T // CH  # 4
NKT = T // 128  # 16 k-tiles

f32 = mybir.dt.float32
f32r = mybir.dt.float32r


def build_nc():
    nc = bacc.Bacc("TRN2", target_bir_lowering=False, debug=False)
    xT = nc.dram_tensor("xT", [C, T], f32r, kind="ExternalInput").ap()
    wqk = nc.dram_tensor("wqk", [C, 2 * F], f32r, kind="ExternalInput").ap()
    wv = nc.dram_tensor("wv", [C, F], f32r, kind="ExternalInput").ap()
    wo = nc.dram_tensor("wo", [F, C], f32r, kind="ExternalInput").ap()
    yT = nc.dram_tensor("yT", [C, T], f32, kind="ExternalOutput").ap()

    with tile.TileContext(nc) as tc:
        with (
            tc.tile_pool(name="consts", bufs=1) as consts,
            tc.tile_pool(name="kv", bufs=1) as kv,
            tc.tile_pool(name="qtp", bufs=2) as qtp,
            tc.tile_pool(name="win", bufs=1) as win,
            tc.tile_pool(name="xin", bufs=1) as xin,
            tc.tile_pool(name="wout", bufs=1) as wout,
            tc.tile_pool(name="obuf", bufs=2) as obuf,
            tc.tile_pool(name="oun", bufs=2) as oun,
            tc.tile_pool(name="pexp", bufs=3) as pexp,
            tc.tile_pool(name="bc", bufs=2) as bc,
            tc.tile_pool(name="tmpb", bufs=2) as tmpb,
            tc.tile_pool(name="ysb", bufs=2) as ysb,
            tc.tile_pool(name="drp", bufs=2, space="DRAM") as drp,
            tc.tile_pool(name="ps", bufs=2, space="PSUM") as ps,
            tc.tile_pool(name="pss", bufs=2, space="PSUM") as pss,
            tc.tile_pool(name="pso", bufs=2, space="PSUM") as pso,
        ):
            tri = consts.tile([128, 128], f32)
            make_upper_triangular(nc, tri[:], val=1.0, diag=True)

            kt_sb = kv.tile([128, 4, T], f32r)  # K^T; head 2p|2p+1 on parts 0-63|64-127
            # V aug ([t, head, d] + ones col) in bf16: stationary operand of the
            # AV matmul only; its error passes linearly into O (~0.2% rel).
            v_sb = kv.tile([128, NKT, HPC, HD + 1], mybir.dt.bfloat16)
            ones = consts.tile([128, NKT * HPC], f32)
            nc.vector.memset(ones[:], 1.0)
            nc.vector.tensor_copy(
                out=v_sb[:, :, :, HD : HD + 1],
                in_=ones[:].rearrange("p (a b c) -> p a b c", a=NKT, b=HPC),
            )

            # weights: split + interleave DMAs per k-tile so the first
            # matmul's operands (wqk[0], xt[0]) land within ~2us
            xt_first = xin.tile([128, 8, CH], f32r, name="xt0", tag="xt")
            wqk_sb = win.tile([128, 8, 2 * F], f32r)
            wv_sb = win.tile([128, 8, F], f32r)
            for kt in range(8):
                nc.sync.dma_start(
                    out=wqk_sb[:, kt, :], in_=wqk[kt * 128 : (kt + 1) * 128, :]
                )
                nc.sync.dma_start(
                    out=xt_first[:, kt, :],
                    in_=xT[kt * 128 : (kt + 1) * 128, 0:CH],
                )
            for kt in range(8):
                nc.sync.dma_start(
                    out=wv_sb[:, kt, :], in_=wv[kt * 128 : (kt + 1) * 128, :]
                )
            wo_sb = wout.tile([128, 4, C], f32r)
            nc.sync.dma_start(
                out=wo_sb[:], in_=wo.rearrange("(ft p) o -> p ft o", p=128)
            )

            qt_tiles = {}
            osb_tiles = {}

            def phase1(j):
                t0 = j * CH
                if j == 0:
                    xt = xt_first
                else:
                    xt = xin.tile([128, 8, CH], f32r, name=f"xt{j}", tag="xt")
                    nc.sync.dma_start(
                        out=xt[:],
                        in_=xT[:, t0 : t0 + CH].rearrange("(kt p) t -> p kt t", p=128),
                    )
                qt = qtp.tile([128, 4, CH], f32r, name=f"qt{j}", tag="qt")
                qt_tiles[j] = qt
                for m in range(8):
                    acc = ps.tile([128, CH], f32, name=f"qk_{j}_{m}", tag="mm")
                    for kt in range(8):
                        nc.tensor.matmul(
                            acc[:],
                            wqk_sb[:, kt, m * 128 : (m + 1) * 128],
                            xt[:, kt, :],
                            start=(kt == 0),
                            stop=(kt == 7),
                        )
                    if m < 4:
                        nc.vector.tensor_copy(out=qt[:, m, :], in_=acc[:])
                    else:
                        nc.vector.tensor_copy(
                            out=kt_sb[:, m % 4, t0 : t0 + CH], in_=acc[:]
                        )
                for s in range(CH // 128):
                    accv = ps.tile([128, F], f32, name=f"v_{j}_{s}", tag="mm")
                    for kt in range(8):
                        nc.tensor.matmul(
                            accv[:],
                            xt[:, kt, s * 128 : (s + 1) * 128],
                            wv_sb[:, kt, :],
                            start=(kt == 0),
                            stop=(kt == 7),
                        )
                    nc.vector.tensor_copy(
                        out=v_sb[:, 4 * j + s, :, 0:HD],
                        in_=accv[:].rearrange("p (h d) -> p h d", h=HPC),
                    )

            def phase2(j):
                q0 = j * CH
                nkt = 4 * j + 4
                o_sb = obuf.tile([128, 4, CH], f32r, name=f"osb{j}", tag="osb")
                osb_tiles[j] = o_sb
                for hp in range(4):
                    o_un = oun.tile(
                        [HD + 1, 2, CH], f32, name=f"oun_{j}_{hp}", tag="oun"
                    )
                    o_ps = [
                        pso.tile([HD + 1, CH], f32, name=f"o_{j}_{hp}_{hf}", tag="o")
                        for hf in range(2)
                    ]
                    def av(kt, p_t, lo):
                        for hf in range(2):
                            h = 2 * hp + hf
                            nc.tensor.matmul(
                                o_ps[hf][:, lo:CH],
                                v_sb[:, kt, h, :],
                                p_t[:, hf, lo:CH],
                                start=(kt == 0),
                                stop=(kt == nkt - 1),
                            )

                    prev = None
                    for kt in range(nkt):
                        k0 = kt * 128
                        lo = max(k0 - q0, 0)
                        # both heads' scores in one 2-bank PSUM tile so a
                        # single wide exp covers the pair; the two K=64
                        # matmuls sit on disjoint PE row groups (0-63 /
                        # 64-127) and are emitted back-to-back so they run
                        # concurrently in the array.
                        s_t = pss.tile(
                            [128, 2, CH], f32, name=f"s_{j}_{hp}_{kt}", tag="s"
                        )
                        p_t = pexp.tile(
                            [128, 2, CH],
                            mybir.dt.bfloat16,
                            name=f"p_{j}_{hp}_{kt}",
                            tag="p",
                        )
                        for hf in range(2):
                            pb = hf * 64
                            nc.tensor.matmul(
                                s_t[:, hf, lo:CH],
                                kt_sb[pb : pb + 64, hp, k0 : k0 + 128],
                                qt_tiles[j][pb : pb + 64, hp, lo:CH],
                                start=True,
                                stop=True,
                            )
                        nc.scalar.activation(
                            out=p_t[:, :, lo:CH],
                            in_=s_t[:, :, lo:CH],
                            func=mybir.ActivationFunctionType.Exp,
                            scale=0.125,
                        )
                        if k0 >= q0:
                            for hf in range(2):
                                nc.vector.tensor_mul(
                                    out=p_t[:, hf, lo : lo + 128],
                                    in0=p_t[:, hf, lo : lo + 128],
                                    in1=tri[:],
                                )
                        # AV runs one k-tile behind so exp(kt) overlaps it
                        if prev is not None:
                            av(*prev)
                        prev = (kt, p_t, lo)
                    av(*prev)
                    for hf in range(2):
                        nc.vector.tensor_copy(
                            out=o_un[:, hf, :], in_=o_ps[hf][:, :]
                        )
                    # normalize this head pair: scatter the 1024 denominators
                    # across 128 partitions via DRAM so reciprocal runs
                    # lane-parallel, then broadcast back per head.
                    rc = drp.tile([2 * CH], f32, name=f"rc{j}_{hp}", tag="rc")
                    dsc = bc.tile([128, 8], f32, name=f"dsc{j}_{hp}", tag="dsc")
                    # SBUF->SBUF partition-scatter: 1024 denominators from
                    # o_un row HD spread across 128 partitions
                    nc.sync.dma_start(out=dsc[:], in_=o_un[HD : HD + 1, :, :])
                    nc.vector.reciprocal(out=dsc[:], in_=dsc[:])
                    nc.sync.dma_start(
                        out=rc[:].rearrange("(p c) -> p c", p=128), in_=dsc[:]
                    )
                    for hf in range(2):
                        bcast = bc.tile(
                            [64, CH], f32, name=f"bb_{j}_{hp}_{hf}", tag="bcast"
                        )
                        nc.sync.dma_start(
                            out=bcast[:],
                            in_=bass.AP(
                                tensor=rc.tensor,
                                offset=rc.offset + hf * CH,
                                ap=[[0, 64], [1, CH]],
                            ),
                        )
                        if hf == 0:
                            nc.vector.tensor_mul(
                                out=o_sb[0:64, hp, :],
                                in0=o_un[0:HD, 0, :],
                                in1=bcast[:],
                            )
                        else:
                            tmp = tmpb.tile(
                                [64, CH], f32r, name=f"tm_{j}_{hp}", tag="tm"
                            )
                            nc.vector.tensor_mul(
                                out=tmp[:], in0=o_un[0:HD, 1, :], in1=bcast[:]
                            )
                            nc.sync.dma_start(out=o_sb[64:128, hp, :], in_=tmp[:])

            def phase3(j):
                q0 = j * CH
                o_sb = osb_tiles[j]
                for ot in range(8):
                    acc = ps.tile([128, CH], f32, name=f"y_{j}_{ot}", tag="mm")
                    for ft in range(4):
                        nc.tensor.matmul(
                            acc[:],
                            wo_sb[:, ft, ot * 128 : (ot + 1) * 128],
                            o_sb[:, ft, :],
                            start=(ft == 0),
                            stop=(ft == 3),
                        )
                    y = ysb.tile([128, CH], f32, name=f"ysb_{j}_{ot}", tag="y")
                    nc.vector.tensor_copy(out=y[:], in_=acc[:])
                    nc.sync.dma_start(
                        out=yT[ot * 128 : (ot + 1) * 128, q0 : q0 + CH], in_=y[:]
                    )

            # interleaved emission; phase3 runs one chunk behind phase2 so the
            # normalization DMA pipeline hides under the next chunk's compute
            phase1(0)
            phase1(1)
            phase2(0)
            phase1(2)
            phase2(1)
            phase3(0)
            phase1(3)
            phase2(2)
            phase3(1)
            phase2(3)
            phase3(2)
            phase3(3)

    nc.compile()
    return nc


def tf32_round(a: np.ndarray) -> np.ndarray:
    """Round fp32 to TF32 (10-bit mantissa), round-to-nearest-even."""
    a = np.ascontiguousarray(a, dtype=np.float32)
    u = a.view(np.uint32)
    r = (u + 0xFFF + ((u >> 13) & 1)) & np.uint32(0xFFFFE000)
    return r.astype(np.uint32).view(np.float32)


def shard_inputs(x, W_qkv, W_out):
    """Build the 8 per-core input maps."""
    xT = [tf32_round(np.ascontiguousarray(x[b].T)) for b in range(B)]
    maps = []
    for core in range(NCORES):
        b, hf = core // 2, core % 2
        wq = W_qkv[:, hf * F : (hf + 1) * F]
        wk = W_qkv[:, C + hf * F : C + (hf + 1) * F]
        wv = W_qkv[:, 2 * C + hf * F : 2 * C + (hf + 1) * F]
        maps.append(
            {
                "xT": xT[b],
                "wqk": tf32_round(np.concatenate([wq, wk], axis=1)),
                "wv": tf32_round(wv),
                "wo": tf32_round(W_out[hf * F : (hf + 1) * F, :]),
            }
        )
    return maps


_NC_CACHE = {}


def get_nc():
    if "nc" not in _NC_CACHE:
        _NC_CACHE["nc"] = build_nc()
    return _NC_CACHE["nc"]


def kernel(x, W_qkv, W_out, _run_kwargs=None):
    x = np.asarray(x, dtype=np.float32)
    W_qkv = np.asarray(W_qkv, dtype=np.float32)
    W_out = np.asarray(W_out, dtype=np.float32)
    nc = get_nc()
    maps = shard_inputs(x, W_qkv, W_out)
    res = run_bass_kernel_spmd(nc, maps, list(range(NCORES)), **(_run_kwargs or {}))
    out = np.empty((B, T, C), dtype=np.float32)
    for b in range(B):
        yT0 = res.results[2 * b]["yT"]
        yT1 = res.results[2 * b + 1]["yT"]
        out[b] = (yT0 + yT1).T
    if _run_kwargs is not None:
        _NC_CACHE["last_results"] = res
    return out
